# revision 1
# baseline (speedup 1.0000x reference)
"""GroupNorm + full spatial self-attention block on 8 Trainium2 NeuronCores.

Strategy: pure data parallelism over batch (B=32 -> 4 images per core, zero
collectives). Per image, everything stays on-chip:

  group-norm (bn_stats over a (image,group)-partition layout) ->
  q/k/v projections (bf16 matmuls) ->
  scores computed TRANSPOSED (S_T[q,p] = k'^T q') so no big transpose is
  ever needed; softmax-without-max (scores are ~N(0,1), exp is safe);
  column sums via a ones-vector matmul; normalization deferred past the
  output projection (the per-query softmax denominator commutes with the
  channel contraction), applied as one broadcast multiply at the end.

Numerics: matmuls run in bf16 (inputs are ~unit-scale; the final nin weight
carries a 0.05 factor, so attention-path rounding lands ~1e-3 relative on the
output). GroupNorm, softmax sums, and the residual path stay fp32.
"""

import numpy as np
import ml_dtypes

import concourse.bass as bass
import concourse.tile as tile
from concourse import mybir
from concourse.vector_clock import ScopedClock
import concourse.bass2jax as _bass2jax
import json as _json

F32 = mybir.dt.float32
F32R = mybir.dt.float32r
BF16 = mybir.dt.bfloat16
AF = mybir.ActivationFunctionType
OP = mybir.AluOpType

B, C, H, W = 32, 512, 32, 32
HW = H * W                      # 1024 spatial positions
NCORES = 8
BL = B // NCORES                # 4 images per core
G = 32                          # groups
GS = C // G                     # 16 channels per group
EPS = 1e-5
P = 128
KC = C // P                     # 4 channel chunks
QT = HW // P                    # 8 query tiles
NH = HW // 512                  # 2 matmul halves of the spatial dim
SCALE = float(C) ** -0.5


# ---------------------------------------------------------------------------
# Workarounds for this walrus build, which encodes at most ONE sync wait per
# instruction. (1) Tile's exit path piles every final sem wait onto a single
# Drain; emit standalone waits instead. (2) Split any remaining multi-wait
# instruction in the BIR into standalone EventSemaphore waits.

def _patched_drain_and_barrier(self, tick_clock, wait_clock):
    nc = self.nc
    probe = nc.sync.nop(nofuse=True)
    wait_clock.add_sem_waits(probe.ins, ScopedClock({None: tick_clock.global_clock}))
    si = probe.ins.sync_info
    waits = list(si.on_wait) if si is not None else []
    if si is not None:
        probe.ins.sync_info = mybir.SyncInfo(on_wait=[], on_update=list(si.on_update))
    name2sem = {s.name: s for s in self.sems.allocated().values()}
    for w in waits:
        nc.sync.wait_ge(name2sem[w.ant_name], w.wait_value)
    nc.sync.drain()
    # the standalone waits above already cover every processor's final tick,
    # so sequencer-only barriers suffice (skips the per-engine drains of the
    # full EVSEM butterfly, ~5-10us of kernel tail)
    nc.all_engine_barrier(sem_only=True)
    popped = nc._tile_sem_poison_stack.pop()
    assert popped is self._sem_poison
    nc.clear_and_free_semaphores(list(self.sems.allocated().values()))
    nc.all_engine_barrier(sem_only=True)


tile.TileContext._drain_and_barrier = _patched_drain_and_barrier

_orig_compile_bir_kernel = _bass2jax.compile_bir_kernel


def _split_multiwait_bir(bir_bytes):
    bir = _json.loads(bir_bytes)
    for fn in bir.get("functions", []):
        for blk in fn.get("blocks", []):
            insts = blk.get("instructions")
            if not insts:
                continue
            out = []
            for ins in insts:
                si = ins.get("sync_info")
                waits = (si or {}).get("on_wait") or []
                if len(waits) > 1:
                    for j, w in enumerate(waits[:-1]):
                        out.append({
                            "debug": ins.get("debug"),
                            "engine": ins["engine"],
                            "ins": [],
                            "outs": [],
                            "name": f"{ins['name']}-xw{j}",
                            "opcode": "EventSemaphore",
                            "sync_info": {"on_update": [], "on_wait": [w]},
                        })
                    si["on_wait"] = [waits[-1]]
                out.append(ins)
            blk["instructions"] = out
    return _json.dumps(bir).encode()


def _compile_bir_kernel_splitwaits(ant_bir_str, compile_dir_path, **kwargs):
    return _orig_compile_bir_kernel(
        _split_multiwait_bir(ant_bir_str), compile_dir_path, **kwargs
    )


_bass2jax.compile_bir_kernel = _compile_bir_kernel_splitwaits


# ---------------------------------------------------------------------------

def _build_program():
    nc = bass.Bass()
    xs = nc.dram_tensor("xs", [BL, C, HW], F32, kind="ExternalInput")
    wq = nc.dram_tensor("wq", [P, KC, C], BF16, kind="ExternalInput")
    wk = nc.dram_tensor("wk", [P, KC, C], BF16, kind="ExternalInput")
    wv = nc.dram_tensor("wv", [P, KC, C], BF16, kind="ExternalInput")
    wn = nc.dram_tensor("wn", [P, KC, C], BF16, kind="ExternalInput")
    bqd = nc.dram_tensor("bq", [C], F32, kind="ExternalInput")
    bkd = nc.dram_tensor("bk", [C], F32, kind="ExternalInput")
    bnd = nc.dram_tensor("bneff", [C], F32, kind="ExternalInput")
    bones_d = nc.dram_tensor("bones", [P, KC, G], BF16, kind="ExternalInput")
    out = nc.dram_tensor("out", [BL, C, HW], F32, kind="ExternalOutput")

    with tile.TileContext(nc) as tc:
        with (
            tc.tile_pool(name="const", bufs=1) as constp,
            tc.tile_pool(name="sb1", bufs=1) as sb1,
            tc.tile_pool(name="sb2", bufs=2) as sb2,
            tc.tile_pool(name="sb3", bufs=3) as sb3,
            tc.tile_pool(name="rows", bufs=1) as rows,
            tc.tile_pool(name="dram", bufs=2, space="DRAM") as dramp,
        ):
            eps_t = constp.tile([P, 1], F32, tag="eps", name="eps")
            nc.vector.memset(eps_t[:], EPS)

            # ---- image 0: x first, then group-norm stats on the (idle) PE:
            # group sums of x and x^2 via a block-ones matmul, so projections
            # can start ~16us in instead of waiting for the 25us DVE
            # bn_stats chain over all images.
            xch0 = []
            for q in range(KC):
                xt = sb2.tile([P, HW], F32, tag=f"xch{q}", name=f"xch{q}")
                nc.sync.dma_start(xt[:], xs[0, P * q:P * (q + 1), :])
                xch0.append(xt)
            bones = constp.tile([P, KC, G], BF16, tag="bones", name="bones")
            nc.sync.dma_start(bones[:], bones_d[:])

            st0 = []
            with tc.tile_pool(name="psG", bufs=1, space="PSUM") as psG:
                xb, x2 = [], []
                for q in range(KC):
                    xbt = sb1.tile([P, HW], BF16, tag=f"xb_{q}", name=f"xb_{q}")
                    nc.vector.tensor_copy(out=xbt[:], in_=xch0[q][:])
                    xb.append(xbt)
                    x2t = sb1.tile([P, HW], BF16, tag=f"x2_{q}", name=f"x2_{q}")
                    nc.scalar.activation(out=x2t[:], in_=xch0[q][:],
                                         func=AF.Square)
                    x2.append(x2t)
                gps = [psG.tile([G, 512], F32, tag=f"g{j}", name=f"g{j}")
                       for j in range(4)]  # x h0, x h1, x2 h0, x2 h1
                for q in range(KC):
                    lhsT = bones[:, q, :]
                    for h_ in range(NH):
                        nc.tensor.matmul(
                            gps[h_][:], lhsT,
                            xb[q][:, 512 * h_:512 * (h_ + 1)],
                            start=(q == 0), stop=(q == KC - 1))
                        nc.tensor.matmul(
                            gps[2 + h_][:], lhsT,
                            x2[q][:, 512 * h_:512 * (h_ + 1)],
                            start=(q == 0), stop=(q == KC - 1))
                red = rows.tile([G, 4], F32, tag="red", name="red")
                for j in range(4):
                    nc.vector.reduce_sum(out=red[:, j:j + 1], in_=gps[j][:],
                                         axis=mybir.AxisListType.X)
                mv0 = rows.tile([G, 2], F32, tag="mv0", name="mv0")
                # mean = (h0+h1)/NGRP ; E[x2] likewise
                nc.vector.tensor_tensor(mv0[:, 0:1], red[:, 0:1], red[:, 1:2],
                                        OP.add)
                nc.vector.tensor_tensor(mv0[:, 1:2], red[:, 2:3], red[:, 3:4],
                                        OP.add)
                nc.vector.tensor_scalar_mul(mv0[:], mv0[:], 1.0 / (GS * HW))
                msq = rows.tile([G, 1], F32, tag="msq", name="msq")
                nc.vector.tensor_tensor(msq[:], mv0[:, 0:1], mv0[:, 0:1],
                                        OP.mult)
                var0 = rows.tile([G, 1], F32, tag="var0", name="var0")
                nc.vector.tensor_tensor(var0[:], mv0[:, 1:2], msq[:],
                                        OP.subtract)
                std0 = rows.tile([G, 1], F32, tag="std0", name="std0")
                nc.scalar.activation(out=std0[:], in_=var0[:], func=AF.Sqrt,
                                     bias=eps_t[:G])
                gnst0 = rows.tile([G, 2], F32, tag="gnst0", name="gnst0")
                nc.vector.reciprocal(out=gnst0[:, 0:1], in_=std0[:])
                nc.vector.tensor_scalar(out=gnst0[:, 1:2], in0=mv0[:, 0:1],
                                        scalar1=gnst0[:, 0:1], scalar2=-1.0,
                                        op0=OP.mult, op1=OP.mult)
                gnexp0 = rows.tile([G, GS, 2], F32, tag="gnexp0", name="gnexp0")
                nc.vector.tensor_copy(
                    out=gnexp0[:],
                    in_=gnst0[:, None, :].to_broadcast((G, GS, 2)))
                for q in range(KC):
                    st = sb2.tile([P, 2], F32, tag=f"st{q}", name=f"st{q}")
                    nc.sync.dma_start(st[:], gnexp0[8 * q: 8 * q + 8, :, :])
                    st0.append(st)

            # ---- resident weights (queued behind image 0's x) ----
            wsb = {}
            for name, dram in (("wq", wq), ("wk", wk), ("wv", wv), ("wn", wn)):
                t = constp.tile([P, KC, C], BF16, tag=f"w_{name}", name=f"w_{name}")
                nc.sync.dma_start(t[:], dram[:])
                wsb[name] = t
            bq_sb = constp.tile([P, KC], F32, tag="bq", name="bq")
            nc.sync.dma_start(bq_sb[:], bqd[:].rearrange("(kc p) -> p kc", p=P))
            bk_sb = constp.tile([P, KC], F32, tag="bk", name="bk")
            nc.sync.dma_start(bk_sb[:], bkd[:].rearrange("(kc p) -> p kc", p=P))
            bn_sb = constp.tile([P, KC], F32, tag="bneff", name="bneff")
            nc.sync.dma_start(bn_sb[:], bnd[:].rearrange("(kc p) -> p kc", p=P))
            ones_col = constp.tile([P, 1], BF16, tag="ones", name="ones")
            nc.vector.memset(ones_col[:], 1.0)

            # ---- images 1-3: bn_stats over the (image,group) layout on DVE,
            # hidden under image 0's compute (their stats aren't needed until
            # ~70us in). Rows 0..32 (image 0) are computed but unused.
            gn_blk = xs[:].rearrange("b (g j) s -> (b g) (j s)", j=GS) \
                          .rearrange("p (i u) -> p i u", u=512)
            gnexp = constp.tile([P, GS, 2], F32, tag="gnexp", name="gnexp")
            with tc.tile_pool(name="gn", bufs=2) as gnp:
                stats = gnp.tile([P, G, 6], F32, tag="stats", name="stats")
                for c8 in range(8):
                    x_blk = gnp.tile([P, 4, 512], F32, tag="x_blk", name="x_blk")
                    nc.sync.dma_start(x_blk[:], gn_blk[:, 4 * c8:4 * (c8 + 1), :])
                    for j in range(4):
                        i = 4 * c8 + j
                        nc.vector.bn_stats(out=stats[:, i, :], in_=x_blk[:, j, :])
                mv = gnp.tile([P, 2], F32, tag="mv", name="mv")
                nc.vector.bn_aggr(out=mv[:], in_=stats[:])
                std = gnp.tile([P, 1], F32, tag="std", name="std")
                nc.scalar.activation(out=std[:], in_=mv[:, 1:2], func=AF.Sqrt,
                                     bias=eps_t[:])
                gnst = gnp.tile([P, 2], F32, tag="gnst", name="gnst")
                nc.vector.reciprocal(out=gnst[:, 0:1], in_=std[:])
                nc.vector.tensor_scalar(out=gnst[:, 1:2], in0=mv[:, 0:1],
                                        scalar1=gnst[:, 0:1], scalar2=-1.0,
                                        op0=OP.mult, op1=OP.mult)
                nc.vector.tensor_copy(
                    out=gnexp[:],
                    in_=gnst[:, None, :].to_broadcast((P, GS, 2)))

            with (
                tc.tile_pool(name="psA", bufs=6, space="PSUM") as psA,
                tc.tile_pool(name="psS", bufs=1, space="PSUM") as psS,
            ):
                for b in range(BL):
                    _image(nc, tc, b, xs, out, wsb, bq_sb, bk_sb, bn_sb,
                           ones_col, gnexp, xch0, st0,
                           sb1, sb2, sb3, rows, dramp, psA, psS)

    return nc


def _image(nc, tc, b, xs, out, wsb, bq_sb, bk_sb, bn_sb, ones_col, gnexp,
           xch0, st0, sb1, sb2, sb3, rows, dramp, psA, psS):
    # x (channel layout, kept for the residual) + per-channel gn scale/shift
    if b == 0:
        xch, st = xch0, st0
    else:
        xch, st = [], []
        for q in range(KC):
            xt = sb2.tile([P, HW], F32, tag=f"xch{q}", name=f"xch{q}")
            nc.sync.dma_start(xt[:], xs[b, P * q:P * (q + 1), :])
            xch.append(xt)
        for q in range(KC):
            s = sb2.tile([P, 2], F32, tag=f"st{q}", name=f"st{q}")
            nc.sync.dma_start(
                s[:], gnexp[b * G + 8 * q: b * G + 8 * q + 8, :, :])
            st.append(s)
    hsb = []
    for q in range(KC):
        ht = sb2.tile([P, HW], BF16, tag=f"h{q}", name=f"h{q}")
        if q % 2 == 0:
            nc.vector.tensor_scalar(out=ht[:], in0=xch[q][:],
                                    scalar1=st[q][:, 0:1],
                                    scalar2=st[q][:, 1:2],
                                    op0=OP.mult, op1=OP.add)
        else:
            # same affine on the scalar engine so the two halves of the
            # normalize run concurrently (it gates the first projection)
            nc.scalar.activation(out=ht[:], in_=xch[q][:], func=AF.Identity,
                                 scale=st[q][:, 0:1], bias=st[q][:, 1:2])
        hsb.append(ht)

    # q' = Wq^T h + bq, k' = Wk^T h + bk   (layout [c_out, p])
    qsb, ksb = [], []
    for wname, outl, bias in (("wq", qsb, bq_sb), ("wk", ksb, bk_sb)):
        w = wsb[wname]
        for m in range(KC):
            dst = sb1.tile([P, HW], BF16, tag=f"{wname}o{m}", name=f"{wname}o{m}")
            ps = [psA.tile([P, 512], F32, tag="mm", name="mm")
                  for _ in range(NH)]
            for k in range(KC):
                lhsT = w[:, k, P * m:P * (m + 1)]
                for h_ in range(NH):
                    nc.tensor.matmul(
                        ps[h_][:], lhsT,
                        hsb[k][:, 512 * h_:512 * (h_ + 1)],
                        start=(k == 0), stop=(k == KC - 1))
            for h_ in range(NH):
                nc.scalar.activation(
                    out=dst[:, 512 * h_:512 * (h_ + 1)],
                    in_=ps[h_][:], func=AF.Identity,
                    bias=bias[:, m:m + 1])
            outl.append(dst)

    # v^T = h^T Wv   (layout [q, c]; bv is folded into bneff on the host)
    vT = []
    for i in range(QT):
        dst = sb1.tile([P, C], BF16, tag=f"v{i}", name=f"v{i}")
        ps = psA.tile([P, 512], F32, tag="mm", name="mm")
        for k in range(KC):
            nc.tensor.matmul(ps[:], hsb[k][:, P * i:P * (i + 1)],
                             wsb["wv"][:, k, :],
                             start=(k == 0), stop=(k == KC - 1))
        nc.scalar.copy(out=dst[:], in_=ps[:])
        vT.append(dst)

    # scores S_T[q,p] = k'^T q' ; E = exp(S_T/sqrt(C)); denominators via
    # ones^T E, lagged one tile behind so the PE never waits on the exp
    Esb = [sb1.tile([P, HW], BF16, tag=f"E{i}", name=f"E{i}") for i in range(QT)]
    sums_ps = [psS.tile([1, 512], F32, tag=f"sums{h_}", name=f"sums{h_}")
               for h_ in range(NH)]

    def scores_i(i):
        ps = [psA.tile([P, 512], F32, tag="mm", name="mm") for _ in range(NH)]
        for k in range(KC):
            lhsT = ksb[k][:, P * i:P * (i + 1)]
            for h_ in range(NH):
                nc.tensor.matmul(
                    ps[h_][:], lhsT,
                    qsb[k][:, 512 * h_:512 * (h_ + 1)],
                    start=(k == 0), stop=(k == KC - 1))
        for h_ in range(NH):
            nc.scalar.activation(
                out=Esb[i][:, 512 * h_:512 * (h_ + 1)],
                in_=ps[h_][:], func=AF.Exp, scale=SCALE)

    def rowsum_i(i):
        for h_ in range(NH):
            nc.tensor.matmul(
                sums_ps[h_][:], ones_col[:],
                Esb[i][:, 512 * h_:512 * (h_ + 1)],
                start=(i == 0), stop=(i == QT - 1))

    for i in range(QT):
        scores_i(i)
    for i in range(QT):
        rowsum_i(i)

    # softmax denominators -> reciprocal (in a [128,8] layout: DVE recip is
    # ~6 cyc/element, a 1-lane row would cost 6+us) -> broadcast via DRAM.
    # Issued before apply so R is ready by the time the epilogue needs it.
    srow = rows.tile([1, HW], F32, tag="srow", name="srow")
    for h_ in range(NH):
        nc.scalar.copy(out=srow[:, 512 * h_:512 * (h_ + 1)],
                       in_=sums_ps[h_][:])
    rscrA = dramp.tile([1, HW], F32, tag="rscrA", name="rscrA")
    nc.sync.dma_start(rscrA[:], srow[:])
    sblk = rows.tile([P, 8], F32, tag="sblk", name="sblk")
    nc.sync.dma_start(sblk[:], rscrA[:])
    rblk = rows.tile([P, 8], F32, tag="rblk", name="rblk")
    nc.vector.reciprocal(out=rblk[:], in_=sblk[:])
    rscrB = dramp.tile([1, HW], F32, tag="rscrB", name="rscrB")
    nc.sync.dma_start(rscrB[:], rblk[:])
    R_sb = sb1.tile([P, HW], F32, tag="Rsb", name="Rsb")
    nc.sync.dma_start(R_sb[:], rscrB[:].partition_broadcast(P))

    # unnormalized attention output hA[c,p] = sum_q v^T[q,c] E[q,p]
    hA = []
    for m in range(KC):
        dst = sb1.tile([P, HW], BF16, tag=f"hA{m}", name=f"hA{m}")
        ps = [psA.tile([P, 512], F32, tag="mm", name="mm") for _ in range(NH)]
        for iq in range(QT):
            lhsT = vT[iq][:, P * m:P * (m + 1)]
            for h_ in range(NH):
                nc.tensor.matmul(
                    ps[h_][:], lhsT,
                    Esb[iq][:, 512 * h_:512 * (h_ + 1)],
                    start=(iq == 0), stop=(iq == QT - 1))
        for h_ in range(NH):
            nc.scalar.copy(out=dst[:, 512 * h_:512 * (h_ + 1)],
                           in_=ps[h_][:])
        hA.append(dst)

    # output projection + epilogue: out = (Wn^T hA) * R + x + bneff
    for m in range(KC):
        osb = sb3.tile([P, HW], F32, tag="osb", name="osb")
        ps = [psA.tile([P, 512], F32, tag="mm", name="mm") for _ in range(NH)]
        for k in range(KC):
            lhsT = wsb["wn"][:, k, P * m:P * (m + 1)]
            for h_ in range(NH):
                nc.tensor.matmul(
                    ps[h_][:], lhsT,
                    hA[k][:, 512 * h_:512 * (h_ + 1)],
                    start=(k == 0), stop=(k == KC - 1))
        for h_ in range(NH):
            sl = slice(512 * h_, 512 * (h_ + 1))
            t1 = sb3.tile([P, 512], F32, tag="t1", name="t1")
            nc.vector.tensor_tensor(t1[:], ps[h_][:], R_sb[:, sl], OP.mult)
            t2 = sb3.tile([P, 512], F32, tag="t2", name="t2")
            nc.vector.tensor_tensor(t2[:], t1[:], xch[m][:, sl], OP.add)
            nc.scalar.activation(out=osb[:, sl], in_=t2[:],
                                 func=AF.Identity,
                                 bias=bn_sb[:, m:m + 1])
        nc.sync.dma_start(out[b, P * m:P * (m + 1), :], osb[:])


_cached_nc = None


def _get_program():
    global _cached_nc
    if _cached_nc is None:
        _cached_nc = _build_program()
    return _cached_nc


def _run(inputs, trace=False, trace_cores=None):
    """Shard, run on 8 cores, gather. Returns (out [B,C,H,W] f32, exec_ns)."""
    from concourse.bass_utils import run_bass_kernel_spmd

    x = np.asarray(inputs["x"], dtype=np.float32).reshape(B, C, HW)
    bf = ml_dtypes.bfloat16

    def shuf(w):
        # [C, C] -> [P, KC, C] so each partition's weight bytes are one
        # contiguous DRAM run (128 descriptors instead of 512)
        w = np.asarray(w, dtype=np.float32).astype(bf)
        return np.ascontiguousarray(w.reshape(KC, P, C).transpose(1, 0, 2))

    wq = shuf(inputs["Wq"])
    wk = shuf(inputs["Wk"])
    wv = shuf(inputs["Wv"])
    wn = shuf(inputs["Wn"])
    bq = np.asarray(inputs["bq"], dtype=np.float32)
    bk = np.asarray(inputs["bk"], dtype=np.float32)
    bv = np.asarray(inputs["bv"], dtype=np.float32)
    bn = np.asarray(inputs["bn"], dtype=np.float32)
    wn32 = np.asarray(inputs["Wn"], dtype=np.float32)
    bneff = (wn32.T @ bv + bn).astype(np.float32)

    bones = np.zeros((P, KC, G), dtype=ml_dtypes.bfloat16)
    for p in range(P):
        for q in range(KC):
            bones[p, q, 8 * q + p // GS] = 1.0
    shared = {"wq": wq, "wk": wk, "wv": wv, "wn": wn,
              "bq": bq, "bk": bk, "bneff": bneff, "bones": bones}
    in_maps = []
    for i in range(NCORES):
        m = dict(shared)
        m["xs"] = np.ascontiguousarray(x[BL * i:BL * (i + 1)])
        in_maps.append(m)

    nc = _get_program()
    kwargs = {}
    if trace:
        kwargs["trace"] = True
        if trace_cores is not None:
            kwargs["trace_cores"] = trace_cores
    res = run_bass_kernel_spmd(nc, in_maps, core_ids=list(range(NCORES)),
                               **kwargs)
    outs = [res.results[i]["out"] for i in range(NCORES)]
    full = np.concatenate(outs, axis=0).reshape(B, C, H, W)
    return full.astype(np.float32), res.exec_time_ns


def kernel(**inputs):
    out, _ = _run(inputs, trace=False)
    return out

